# revision 21
# baseline (speedup 1.0000x reference)
"""CrossAttention Trainium2 Bass kernel.

Problem: y = CrossAttention(x, kv) with the reference's no-transpose q-reshape
quirk, B=8, N=1024, C=768, H=8, D=96.

Strategy: pure data parallelism — batch element b on NeuronCore b. Host
pre-transposes x/kv/weights so every matmul contraction dim lands on SBUF
partitions. All matmuls run with bf16 operands (fp32 PSUM accumulate):
fp32r and fp16 measure ~2 cycles/row on HW for 512-wide moving dims while
bf16 runs at 1 cycle/row; bf16 keeps rel err ~6e-3 (gate is 2e-2).

Schedule (software-pipelined around the ACT exp floor of ~8.3us/head):
  1. Q-u0 wave (cb-outer, streams against arriving DMAs)
  2. K heads 0-3, V-u0 group-serial
  3. attention h=0..7; the remaining 24 projection groups (Q-u1, K h4-7,
     V-u1) are fed as fillers between S-matmuls so PE stays busy while the
     psum ring waits on ACT exp.
  4. output projection split: heads 0-5/0-6 accumulate into f32 SBUF staging
     (ysbA) as soon as their norms land; head 7's normalization is deferred
     into the final fused op: ysb = psum7 * (1/rowsum7)[per-partition]
     + ysbA, with the bias riding the rowsum row (rs*bias/rs = bias).
  Per-head softmax normalization: the ones column of V makes each PV matmul
  also emit the row-sums (row 96); heads 0-6 are normalized in place via a
  DMA round-trip that repartitions 1/rowsum; head 7 skips normalization
  entirely (deferred into yproj B).
  Tail tensor ops are spread across DVE and GpSimd (Pool) so no single
  engine serializes the ending.
"""
import sys
sys.path.insert(0, '/opt/trn_rl_repo')

import numpy as np
import concourse.bass as bass
import concourse.mybir as mybir
import concourse.tile as tile
from concourse.bass_utils import run_bass_kernel_spmd

F32 = mybir.dt.float32
F16 = mybir.dt.float16
BF16 = mybir.dt.bfloat16
import os as _os
if _os.environ.get("KERNEL_DT16", "bf16") == "bf16":
    import ml_dtypes as _mld
    DT16, NP16 = BF16, _mld.bfloat16
else:
    DT16, NP16 = F16, np.float16
AF = mybir.ActivationFunctionType

B, N, C = 8, 1024, 768
H, D = 8, 96
SCALE = D ** -0.5
NB = N // 128   # 8 n-blocks
CB = C // 128   # 6 c-blocks
HN = H * N      # 8192


def _legalize_waits(nc, max_waits=1):
    """This container's walrus accepts at most one sync-wait command per
    instruction; move excess waits onto preceding NoOps on the same engine."""
    ctr = 0
    for f in nc.m.functions:
        for blk in f.blocks:
            out = []
            changed = False
            for ins in blk.instructions:
                si = ins.sync_info
                waits = list(si.on_wait) if si is not None and si.on_wait else []
                if len(waits) > max_waits:
                    changed = True
                    for w in waits[:-max_waits]:
                        ctr += 1
                        nop = mybir.InstNoOp(name=f"I-wsplit-{ctr}")
                        nop.engine = ins.engine
                        nop.sync_info = mybir.SyncInfo(on_wait=[w], on_update=[])
                        out.append(nop)
                    ins.sync_info = mybir.SyncInfo(
                        on_wait=waits[-max_waits:],
                        on_update=list(si.on_update or []))
                out.append(ins)
            if changed:
                blk.instructions = out
    return ctr


def build_kernel(repeat=1):
    nc = bass.Bass('TRN2', target_bir_lowering=False, debug=False, num_devices=B)

    xT = nc.dram_tensor("xT", [C, N], DT16, kind="ExternalInput").ap()
    kvT = nc.dram_tensor("kvT", [C, N], DT16, kind="ExternalInput").ap()
    WqT = nc.dram_tensor("WqT", [C, C], DT16, kind="ExternalInput").ap()
    WkvT = nc.dram_tensor("WkvT", [C, 2 * C], DT16, kind="ExternalInput").ap()
    WpjT = nc.dram_tensor("WpjT", [C, C], DT16, kind="ExternalInput").ap()
    bias = nc.dram_tensor("bias", [1, C], DT16, kind="ExternalInput").ap()
    y = nc.dram_tensor("y", [N, C], DT16, kind="ExternalOutput").ap()
    rs_dram = nc.dram_tensor("rs_scratch", [1, HN], DT16, kind="Internal").ap()
    ri_dram = nc.dram_tensor("ri_scratch", [1, HN], DT16, kind="Internal").ap()

    with tile.TileContext(nc) as tc:
      for _rep in range(repeat):
        with tc.tile_pool(name="persist", bufs=1) as pp, \
             tc.tile_pool(name="norm", bufs=1) as pn, \
             tc.tile_pool(name="pt", bufs=10) as ppt, \
             tc.tile_pool(name="yout", bufs=8) as py, \
             tc.tile_pool(name="psum_mm", bufs=2, space="PSUM") as pmm, \
             tc.tile_pool(name="psum_o", bufs=4, space="PSUM") as pso:
            QT = pp.tile([D, HN], DT16, tag="QT")
            KT = pp.tile([D, HN], DT16, tag="KT")
            V = [pp.tile([128, H * 97], DT16, tag=f"V{i}", name=f"V{i}")
                 for i in range(NB)]
            kvTs = [pp.tile([128, N], DT16, tag=f"kv{i}", name=f"kvTs{i}")
                    for i in range(CB)]
            WkvTs = [pp.tile([128, 2 * C], DT16, tag=f"Wkv{i}",
                             name=f"WkvTs{i}") for i in range(CB)]
            xTs = [pp.tile([128, N], DT16, tag=f"xT{i}", name=f"xTs{i}")
                   for i in range(CB)]
            WqTs = [pp.tile([128, C], DT16, tag=f"Wq{i}", name=f"WqTs{i}")
                    for i in range(CB)]
            Oall = pp.tile([97, HN], DT16, tag="Oall")
            Wp = []
            for h in range(H):
                rows = 97 if h == H - 1 else 96
                Wp.append(pp.tile([rows, C], DT16, tag=f"Wp{h}",
                                  name=f"Wp{h}"))
            ysbA = [pn.tile([128, C], DT16, tag=f"yA{nb}", name=f"ysbA{nb}")
                    for nb in range(NB)]

            # Input DMAs: one serial queue in issue order, so order is the
            # prioritization. Q inputs first (first consumer), then kv/Wkv,
            # then Wproj/bias (consumed last).
            for i in range(CB):
                nc.sync.dma_start(WqTs[i][:], WqT[128 * i:128 * (i + 1), :])
                nc.sync.dma_start(xTs[i][:, 0:512],
                                  xT[128 * i:128 * (i + 1), 0:512])
            for i in range(CB):
                nc.sync.dma_start(WkvTs[i][:], WkvT[128 * i:128 * (i + 1), :])
                nc.sync.dma_start(kvTs[i][:], kvT[128 * i:128 * (i + 1), :])
            for i in range(CB):
                nc.sync.dma_start(xTs[i][:, 512:1024],
                                  xT[128 * i:128 * (i + 1), 512:1024])
            for h in range(H):
                nc.sync.dma_start(Wp[h][0:96, :], WpjT[96 * h:96 * (h + 1), :])
            nc.sync.dma_start(Wp[H - 1][96:97, :], bias[:])

            ones_stage = pp.tile([128, 8], F32, tag="ones")
            nc.vector.memset(ones_stage[:], 1.0)
            for nb in range(NB):
                ones_cols = V[nb][:].rearrange(
                    "p (h c) -> p h c", h=H)[:, :, 96:97]
                nc.vector.tensor_copy(ones_cols, ones_stage[:])

            # ---- projection helpers ----
            def q_evac(g, ps):
                r, u = g
                dest = QT[:].rearrange(
                    "p (h j r) -> p h j r", h=H, j=128)[
                    :, 4 * u:4 * (u + 1), :, r:r + 1]
                nc.vector.tensor_copy(dest, ps)

            def k_evac(g, ps):
                h, u = g
                nc.vector.tensor_copy(
                    KT[:, 1024 * h + 512 * u:1024 * h + 512 * (u + 1)], ps)

            def v_evac(g, ps):
                nb, u = g
                dest = V[nb][:].rearrange(
                    "p (h c) -> p h c", h=H)[:, 4 * u:4 * (u + 1), 0:96]
                nc.vector.tensor_copy(dest, ps)

            q_lhsT = lambda g, cb: WqTs[cb][:, 96 * g[0]:96 * (g[0] + 1)]
            q_rhs = lambda g, cb: xTs[cb][:, 512 * g[1]:512 * (g[1] + 1)]
            k_lhsT = lambda g, cb: WkvTs[cb][:, 96 * g[0]:96 * (g[0] + 1)]
            k_rhs = lambda g, cb: kvTs[cb][:, 512 * g[1]:512 * (g[1] + 1)]
            v_lhsT = lambda g, cb: kvTs[cb][:, 128 * g[0]:128 * (g[0] + 1)]
            v_rhs = lambda g, cb: WkvTs[cb][:, C + 384 * g[1]:
                                            C + 384 * (g[1] + 1)]

            _sv = [0]

            def serial_group(g, lhsT_of, rhs_of, evac, mm_parts, ncols):
                _sv[0] += 1
                ps = pso.tile([128, 512], F32, tag="po",
                              name=f"ser{_sv[0]}")
                ps = ps[0:mm_parts, 0:ncols]
                for cb in range(CB):
                    nc.tensor.matmul(ps, lhsT_of(g, cb), rhs_of(g, cb),
                                     start=(cb == 0), stop=(cb == CB - 1))
                evac(g, ps)

            # P2: Q-u0 wave (cb outer so PE consumes tiles as DMAs deliver;
            # 8 concurrent psum groups; pso slots evacuate first since the
            # serial phase allocates from the same ring)
            _wv = [0]

            def proj_wave(groups, lhsT_of, rhs_of, evac, mm_parts=D):
                for i in range(0, len(groups), 8):
                    wave = groups[i:i + 8]
                    _wv[0] += 1
                    ts = [pmm.tile([128, 1024], F32, tag="mm",
                                   name=f"wmm{_wv[0]}_{j}")
                          for j in range(2)]
                    slots = [ts[0][0:mm_parts, 0:512],
                             ts[0][0:mm_parts, 512:1024],
                             ts[1][0:mm_parts, 0:512],
                             ts[1][0:mm_parts, 512:1024]] + [
                        pso.tile([128, 512], F32, tag="po",
                                 name=f"wpo{_wv[0]}_{j}")
                        [0:mm_parts, 0:512] for j in range(4)]
                    for cb in range(CB):
                        for g, ps in zip(wave, slots):
                            nc.tensor.matmul(
                                ps, lhsT_of(g, cb), rhs_of(g, cb),
                                start=(cb == 0), stop=(cb == CB - 1))
                    order = list(zip(wave, slots))
                    for g, ps in order[4:] + order[:4]:
                        evac(g, ps)

            proj_wave([(r, 0) for r in range(8)], q_lhsT, q_rhs, q_evac)

            # P3a: K heads 0-3 as a second streaming wave (each serial group
            # would need all six kv/Wkv cb-tiles up front, but they are still
            # arriving); P4a: V-u0 group-serial (inputs resident by then)
            proj_wave([(h, u) for h in range(4) for u in range(2)],
                      k_lhsT, k_rhs, k_evac)
            for nb in range(NB):
                serial_group((nb, 0), v_lhsT, v_rhs, v_evac, 128, 384)

            # Remaining projection groups become fillers between S-matmuls:
            # Q-u1 first (needed by S(4) emitted at h=3), then K h4-7
            # (needed by S(4..7)), then V-u1 (needed by PV(4) at h=4).
            fillers = []
            for r in range(8):
                fillers.append(lambda r=r: serial_group(
                    (r, 1), q_lhsT, q_rhs, q_evac, D, 512))
            for h in range(4, H):
                for u in range(2):
                    fillers.append(lambda h=h, u=u: serial_group(
                        (h, u), k_lhsT, k_rhs, k_evac, D, 512))
            for nb in range(NB):
                fillers.append(lambda nb=nb: serial_group(
                    (nb, 1), v_lhsT, v_rhs, v_evac, 128, 384))
            fillers.reverse()   # pop() consumes from the front of the plan

            # ---- attention helpers ----
            def emit_S(h, P_of, kb_start=0):
                for kb in range(kb_start, NB):
                    ps = pmm.tile([128, 1024], F32, tag="mm",
                                  name=f"s{h}_{kb}")
                    for u in range(2):
                        nc.tensor.matmul(
                            ps[:, 512 * u:512 * (u + 1)],
                            KT[:, 1024 * h + 128 * kb:
                               1024 * h + 128 * (kb + 1)],
                            QT[:, 1024 * h + 512 * u:
                               1024 * h + 512 * (u + 1)],
                            start=True, stop=True)
                    nc.scalar.activation(P_of[kb][:], ps[:], AF.Exp)
                    if fillers:
                        fillers.pop()()

            def emit_PV(h, P_of):
                for u in range(2):
                    po = pso.tile([97, 512], F32, tag="po",
                                  name=f"po{h}_{u}")
                    for kb in range(NB):
                        nc.tensor.matmul(
                            po[:], V[kb][:, 97 * h:97 * (h + 1)],
                            P_of[kb][:, 512 * u:512 * (u + 1)],
                            start=(kb == 0), stop=(kb == NB - 1))
                    nc.vector.tensor_copy(
                        Oall[:, 1024 * h + 512 * u:
                             1024 * h + 512 * (u + 1)], po[:])

            def emit_norm(h):
                """rowsum -> 1/rowsum broadcast to all partitions ->
                in-place normalize Oall's head-h slice (DMA round-trip)."""
                sl = slice(1024 * h, 1024 * (h + 1))
                nc.sync.dma_start(rs_dram[0:1, sl], Oall[96:97, sl])
                rsh = pn.tile([128, 8], DT16, tag="rs", name=f"rs{h}", bufs=2)
                nc.sync.dma_start(
                    rsh[:],
                    rs_dram[0:1, sl].rearrange("p (a b) -> (p a) b", a=128))
                rih = pn.tile([128, 8], F32, tag="ri", name=f"ri{h}", bufs=2)
                nc.vector.reciprocal(rih[:], rsh[:])
                rirh = pn.tile([128, 8], DT16, tag="rir", name=f"rir{h}",
                               bufs=2)
                nc.vector.tensor_copy(rirh[:], rih[:])
                nc.sync.dma_start(
                    ri_dram[0:1, sl].rearrange("p (a b) -> (p a) b", a=128),
                    rirh[:])
                bch = pn.tile([97, N], DT16, tag="bc", name=f"bc{h}")
                nc.sync.dma_start(
                    bch[:], bass.AP(ri_dram.tensor, 1024 * h,
                                    [[0, 97], [1, N]]))
                nc.vector.tensor_mul(Oall[:, sl], Oall[:, sl], bch[:])

            # ---- output projection (split A/B, deferred head-7 norm) ----
            def emit_yprojA(nb, nheads):
                for u in range(2):
                    ps = pso.tile([128, 512], F32, tag="po",
                                  name=f"ypA{nb}_{u}")[:, 0:384]
                    for h in range(nheads):
                        nc.tensor.matmul(
                            ps,
                            Oall[0:96, 1024 * h + 128 * nb:
                                 1024 * h + 128 * (nb + 1)],
                            Wp[h][0:96, 384 * u:384 * (u + 1)],
                            start=(h == 0), stop=(h == nheads - 1))
                    nc.vector.tensor_copy(
                        ysbA[nb][:, 384 * u:384 * (u + 1)], ps)

            def emit_yprojA6(nb):
                """Append head 6 into ysbA for the nb-blocks whose A-group
                ran before norm(6) finished."""
                for u in range(2):
                    ps = pso.tile([128, 512], F32, tag="po",
                                  name=f"ypA6_{nb}_{u}")[:, 0:384]
                    nc.tensor.matmul(
                        ps,
                        Oall[0:96, 1024 * 6 + 128 * nb:
                             1024 * 6 + 128 * (nb + 1)],
                        Wp[6][0:96, 384 * u:384 * (u + 1)],
                        start=True, stop=True)
                    nc.vector.tensor_add(
                        ysbA[nb][:, 384 * u:384 * (u + 1)],
                        ysbA[nb][:, 384 * u:384 * (u + 1)], ps)

            def emit_yprojB(nb, invrs7):
                """Head 7 with deferred normalization: the single matmul
                includes the rowsum row (96) times Wp[7]'s bias row, so
                psum * (1/rs) + ysbA = O7norm@W7 + bias + rest."""
                ysb = py.tile([128, C], DT16, tag="ysb", name=f"ysbB{nb}")
                pool = pso if nb % 2 == 0 else pmm
                for u in range(2):
                    ps = pool.tile([128, 512], F32,
                                   tag="po" if nb % 2 == 0 else "mm",
                                   name=f"ypB{nb}_{u}")[:, 0:384]
                    nc.tensor.matmul(
                        ps,
                        Oall[0:97, 1024 * 7 + 128 * nb:
                             1024 * 7 + 128 * (nb + 1)],
                        Wp[7][0:97, 384 * u:384 * (u + 1)],
                        start=True, stop=True)
                    # ACT scales by 1/rowsum (per-partition) out of PSUM;
                    # DVE then adds the staged heads 0-6 in all-16-bit mode
                    sl = slice(384 * u, 384 * (u + 1))
                    nc.scalar.activation(ysb[:, sl], ps, AF.Copy,
                                         scale=invrs7[:, nb:nb + 1])
                    nc.vector.tensor_add(ysb[:, sl], ysb[:, sl],
                                         ysbA[nb][:, sl])
                nc.sync.dma_start(y[128 * nb:128 * (nb + 1), :], ysb[:])

            # ---- attention main loop ----
            P_tiles = {}
            P_tiles[0] = [ppt.tile([128, N], DT16, tag="pt", name=f"P0_{i}")
                          for i in range(NB)]
            emit_S(0, P_tiles[0])
            for h in range(H):
                if h + 1 < H:
                    P_tiles[h + 1] = [
                        ppt.tile([128, N], DT16, tag="pt",
                                 name=f"P{h + 1}_{i}")
                        for i in range(NB)]
                    emit_S(h + 1, P_tiles[h + 1])
                emit_PV(h, P_tiles.pop(h))
                if h == 6:
                    # two A-groups (heads 0-5) bridge the exp(7) ACT window
                    # while norm(6) round-trips
                    emit_yprojA(0, 6)
                    emit_yprojA(1, 6)
                    emit_norm(h)
                elif h == 7:
                    # deferred head-7 normalization: fetch 1/rowsum
                    # per-partition while the A-groups run
                    sl7 = slice(1024 * 7, 1024 * 8)
                    nc.sync.dma_start(rs_dram[0:1, sl7], Oall[96:97, sl7])
                    rsh7 = pn.tile([128, 8], DT16, tag="rs7")
                    nc.sync.dma_start(
                        rsh7[:],
                        rs_dram[0:1, sl7].rearrange(
                            "p (b a) -> (p a) b", a=128))
                    for nb in range(2, NB):
                        emit_yprojA(nb, 7)
                    emit_yprojA6(0)
                    emit_yprojA6(1)
                    invrs7 = pn.tile([128, 8], F32, tag="ri7")
                    nc.vector.reciprocal(invrs7[:], rsh7[:])
                    for nb in range(NB):
                        emit_yprojB(nb, invrs7)
                else:
                    emit_norm(h)

    _legalize_waits(nc)
    return nc


def prep_in_maps(x, kv, Wq, Wkv, Wproj, bproj):
    """Host-side prep: transpose + 16-bit cast, one in_map per core/batch."""
    x = np.asarray(x, dtype=np.float32)
    kv = np.asarray(kv, dtype=np.float32)
    WqTs = (np.ascontiguousarray(np.asarray(Wq, np.float32).T)
            * np.float32(SCALE)).astype(NP16)
    WkvT = np.ascontiguousarray(np.asarray(Wkv, np.float32).T).astype(NP16)
    WpjT = np.ascontiguousarray(np.asarray(Wproj, np.float32).T).astype(NP16)
    bias_np = np.asarray(bproj, np.float32).reshape(1, C).astype(NP16)
    in_maps = []
    for b in range(B):
        in_maps.append({
            "xT": np.ascontiguousarray(x[b].T).astype(NP16),
            "kvT": np.ascontiguousarray(kv[b].T).astype(NP16),
            "WqT": WqTs,
            "WkvT": WkvT,
            "WpjT": WpjT,
            "bias": bias_np,
        })
    return in_maps


_NC_CACHE = {}


def kernel(x, kv, Wq, Wkv, Wproj, bproj, _trace=False):
    in_maps = prep_in_maps(x, kv, Wq, Wkv, Wproj, bproj)
    if "nc" not in _NC_CACHE:
        _NC_CACHE["nc"] = build_kernel()
    nc = _NC_CACHE["nc"]
    res = run_bass_kernel_spmd(nc, in_maps, core_ids=list(range(B)),
                               trace=_trace)
    out = np.stack([r["y"] for r in res.results]).astype(np.float32)
    if _trace:
        return out, res
    return out
